# revision 1
# baseline (speedup 1.0000x reference)
"""Trainium2 Bass kernel for nn_Discourse (BERT span-pool + 2x TransformerConv GNN).

Sharding: data-parallel over docs for the span pooling (8 docs/core);
head-parallel for the graph convs (head h on core h; C=1024 = 8 heads x 128).
Two on-device AllGathers move node features between the two shardings.

Key algebraic simplification: edge_attr @ eW is rank-2 in the edge scalars
(type, is_main):  e_vec(g) = t_g*wr + m_g*wm + wc  with wr = Wr @ eW,
wm = Wm @ eW, wc = (br+bm) @ eW (computed on device).  The k-side edge term
folds into per-node scalars A=q.wr, B=q.wm, C=q.wc; the v-side term folds
into three rank-1 outer products per node block.

Per-edge work uses dst-sorted edges bucketed by 128-node blocks:
dma_gather of node rows (dst payload [q|C|A|B], k rows, v rows), per-edge
logits = (q.k + t*A + m*B + C)/sqrt(d), exp without max-subtraction (|logit|
< 0.1 at this model scale; b2 is dropped since softmax is shift-invariant),
then scatter-softmax/aggregation via one-hot (iota==dst_local)*ex matmuls
accumulated in PSUM.
"""

import math

import numpy as np

import concourse.bacc as bacc
import concourse.bass as bass
import concourse.mybir as mybir
import concourse.tile as tile
from concourse.bass_utils import run_bass_kernel_spmd
from concourse.masks import make_identity

F32 = mybir.dt.float32
F32R = mybir.dt.float32r
BF16 = mybir.dt.bfloat16
I16 = mybir.dt.int16
I32 = mybir.dt.int32

B, E, L, D = 64, 32, 512, 768
C1 = C2 = 1024
H = 8
EG = 32768
SPAN_H = 512
NLAB = 4
N = B * E            # 2048 nodes
NCORES = 8
DPC = B // NCORES    # 8 docs per core
NPC = N // NCORES    # 256 nodes per core (pooling side)
NBLK = N // 128      # 16 node blocks
ISD = 1.0 / math.sqrt(128.0)

AluOp = mybir.AluOpType
ActFn = mybir.ActivationFunctionType


def _wrap_idx(idx, pad_to=None):
    """int16 indices -> [128, n/16] wrapped layout (g at [g%16, g//16]),
    replicated across the 8 gpsimd cores (partition groups of 16)."""
    idx = np.asarray(idx, dtype=np.int16)
    n = idx.shape[0]
    if pad_to is not None and n < pad_to:
        idx = np.concatenate([idx, np.zeros(pad_to - n, np.int16)])
        n = pad_to
    assert n % 16 == 0
    w = idx.reshape(n // 16, 16).T
    return np.tile(w, (8, 1)).copy()


def _edge_prep(batch_edge, batch_edge_type, batch_is_main, ntpb):
    """Sort edges by dst, bucket into 16 blocks of 128 dst nodes, pad each
    block to ntpb tiles of 128 edges. Edge slot s of block b sits at
    [partition s%128, tile b*ntpb + s//128]."""
    src = np.asarray(batch_edge[0], dtype=np.int64)
    dst = np.asarray(batch_edge[1], dtype=np.int64)
    t = np.asarray(batch_edge_type, dtype=np.float32).reshape(-1)
    m = np.asarray(batch_is_main, dtype=np.float32).reshape(-1)

    order = np.argsort(dst, kind="stable")
    src, dst, t, m = src[order], dst[order], t[order], m[order]
    blk = dst // 128

    NT = NBLK * ntpb
    cap = ntpb * 128
    dstloc = np.zeros((128, NT), np.float32)
    tval = np.zeros((128, NT), np.float32)
    mval = np.zeros((128, NT), np.float32)
    vadd = np.full((128, NT), -100000.0, np.float32)
    ets = np.zeros((128, NT, 4), np.float32)
    srcidx_w = np.zeros((128, NT * 8), np.int16)
    dstloc_w = np.zeros((128, NT * 8), np.int16)

    for b in range(NBLK):
        sel = blk == b
        nb = int(sel.sum())
        assert nb <= cap, f"block {b}: {nb} edges > capacity {cap}"
        sl = np.zeros(cap, np.float32)
        sv = np.zeros(cap, np.int16)
        tv = np.zeros(cap, np.float32)
        mv = np.zeros(cap, np.float32)
        dl16 = np.zeros(cap, np.int16)
        sl[:nb] = (dst[sel] - b * 128).astype(np.float32)
        dl16[:nb] = (dst[sel] - b * 128).astype(np.int16)
        sv[:nb] = src[sel].astype(np.int16)
        tv[:nb] = t[sel]
        mv[:nb] = m[sel]
        va = np.full(cap, -100000.0, np.float32)
        va[:nb] = 0.0
        cols = slice(b * ntpb, (b + 1) * ntpb)
        dstloc[:, cols] = sl.reshape(ntpb, 128).T
        tval[:, cols] = tv.reshape(ntpb, 128).T
        mval[:, cols] = mv.reshape(ntpb, 128).T
        vadd[:, cols] = va.reshape(ntpb, 128).T
        ets[:, cols, 0] = 1.0
        ets[:, cols, 1] = tv.reshape(ntpb, 128).T
        ets[:, cols, 2] = mv.reshape(ntpb, 128).T
        srcidx_w[:, b * ntpb * 8:(b + 1) * ntpb * 8] = _wrap_idx(sv)
        dstloc_w[:, b * ntpb * 8:(b + 1) * ntpb * 8] = _wrap_idx(dl16)

    return dict(DSTLOC=dstloc, TVAL=tval, MVAL=mval, VADD=vadd, ETS=ets,
                SRCIDX=srcidx_w, DSTIDX=dstloc_w)


# --------------------------------------------------------------------------
# device program
# --------------------------------------------------------------------------

def build_program(ntpb, repeat=1, inline_inputs=None):
    import os
    stage = os.environ.get("K_STAGE", "full")
    nc = bacc.Bacc("TRN2", target_bir_lowering=False, debug=False)
    NT = NBLK * ntpb

    def din(name, shape, dt):
        if inline_inputs is not None:
            data = np.asarray(inline_inputs[name]).astype(mybir.dt.np(dt))
            assert list(data.shape) == list(shape), (name, data.shape, shape)
            h = nc.inline_tensor(data, name=name)
            if h.dtype != dt:
                h = bass.DRamTensorHandle(h.name, list(shape), dt)
            return h
        return nc.dram_tensor(name, shape, dt, kind="ExternalInput")

    io = dict(
        LHT=din("LHT", [D, DPC * L], F32R),
        LH=din("LH", [DPC * L, D], F32R),
        MASKT=din("MASKT", [E, DPC, L], F32),
        W1=din("W1", [D, SPAN_H], F32R),
        B1C=din("B1C", [128, 4], F32),
        W2=din("W2", [SPAN_H, 1], F32R),
        DSTLOC=din("DSTLOC", [128, NT], F32),
        TVAL=din("TVAL", [128, NT], F32),
        MVAL=din("MVAL", [128, NT], F32),
        VADD=din("VADD", [128, NT], F32),
        ETS=din("ETS", [128, NT, 4], F32R),
        SRCIDX=din("SRCIDX", [128, NT * 8], I16),
        DSTIDX=din("DSTIDX", [128, NT * 8], I16),
        SELIDX=din("SELIDX", [128, 8], I16),
        FCW=din("FCW", [128, NLAB], F32R),
        FCB=din("FCB", [1, NLAB], F32R),
        OUT=nc.dram_tensor("OUT", [B, NLAB], F32, kind="ExternalOutput"),
    )
    for l, inc in ((1, 6), (2, 8)):
        io[f"WQ{l}"] = din(f"WQ{l}", [inc * 128, 128], F32R)
        io[f"QB{l}"] = din(f"QB{l}", [128, 1], F32)
        io[f"WK{l}"] = din(f"WK{l}", [(inc + 1) * 128, 128], F32R)
        io[f"WV{l}"] = din(f"WV{l}", [(inc + 1) * 128, 128], F32R)
        io[f"WS{l}"] = din(f"WS{l}", [(inc + 1) * 128, 128], F32R)
        io[f"SB{l}"] = din(f"SB{l}", [128, 1], F32)
        io[f"E1WS{l}"] = din(f"E1WS{l}", [C1, 128], F32R)
        io[f"WRM4{l}"] = din(f"WRM4{l}", [C1, 4], F32R)

    with tile.TileContext(nc) as tc:
        with (
            tc.tile_pool(name="const", bufs=1) as cp,
            tc.tile_pool(name="ps", bufs=4, space="PSUM") as pp,
            tc.tile_pool(name="acc", bufs=2, space="PSUM") as ap_,
            tc.tile_pool(name="dram", bufs=1, space="DRAM") as dp,
        ):
            g = dict(io)
            g["pp"], g["ap_"], g["cp"], g["tc"] = pp, ap_, cp, tc

            # constants
            ident_f = cp.tile([128, 128], F32, tag="identf", name="identf")
            make_identity(nc, ident_f[:])
            g["ident"] = cp.tile([128, 128], F32R, tag="ident", name="ident")
            nc.vector.tensor_copy(out=g["ident"][:], in_=ident_f[:])
            iota_i = cp.tile([128, 128], I32)
            nc.gpsimd.iota(iota_i[:], pattern=[[1, 128]], base=0,
                           channel_multiplier=0)
            g["iota_f"] = cp.tile([128, 128], F32, tag="iotaf", name="iotaf")
            nc.vector.tensor_copy(out=g["iota_f"][:], in_=iota_i[:])
            ones_f = cp.tile([128, 128], F32, tag="onesf", name="onesf")
            nc.vector.memset(ones_f[:], 0.0)
            nc.vector.memset(ones_f[:1, :], 1.0)
            g["ones_blk"] = cp.tile([128, 128], F32R, tag="onesblk", name="onesblk")
            nc.vector.tensor_copy(out=g["ones_blk"][:], in_=ones_f[:])
            g["ones1"] = cp.tile([1, 128], F32R, tag="ones1", name="ones1")
            nc.vector.tensor_copy(out=g["ones1"][:], in_=ones_f[:1, :])

            # persistent DRAM
            g["AGIN0"] = dp.tile([D, NPC], F32R, tag="AGIN0", name="AGIN0")
            for _r in range(repeat):
                g[f"AGOUT0_{_r}"] = dp.tile([NCORES * D, NPC], F32R, tag=f"AGOUT0{_r}", name=f"AGOUT0{_r}", addr_space="Shared")
            g["AGIN1"] = dp.tile([128, N], F32R, tag="AGIN1", name="AGIN1")
            for _r in range(repeat):
                g[f"AGOUT1_{_r}"] = dp.tile([C1, N], F32R, tag=f"AGOUT1{_r}", name=f"AGOUT1{_r}", addr_space="Shared")
            g["KTAB"] = dp.tile([N, 128], BF16, tag="KTAB", name="KTAB")
            g["VTAB"] = dp.tile([N, 128], F32R, tag="VTAB", name="VTAB")
            g["PAYTAB"] = dp.tile([N, 256], BF16, tag="PAYTAB", name="PAYTAB")
            g["OUT2TAB"] = dp.tile([N, 128], F32R, tag="OUT2TAB", name="OUT2TAB")

            # edge arrays (persistent SBUF)
            for nm, shp, dt_ in (("DSTLOC", [128, NT], F32), ("TVAL", [128, NT], F32),
                                 ("MVAL", [128, NT], F32), ("VADD", [128, NT], F32),
                                 ("ETS", [128, NT, 4], F32R),
                                 ("SRCIDX", [128, NT * 8], I16),
                                 ("DSTIDX", [128, NT * 8], I16),
                                 ("SELIDX", [128, 8], I16)):
                t_ = cp.tile(shp, dt_, tag=f"sb{nm}", name=f"sb{nm}")
                nc.sync.dma_start(out=t_[:], in_=io[nm][:])
                g[f"sb{nm}"] = t_
            for nm, shp, dt_ in (("FCW", [128, NLAB], F32R), ("FCB", [1, NLAB], F32R)):
                t_ = cp.tile(shp, dt_, tag=f"sb{nm}", name=f"sb{nm}")
                nc.sync.dma_start(out=t_[:], in_=io[nm][:])
                g[f"sb{nm}"] = t_

            for rep in range(repeat):
                g["AGOUT0"] = g[f"AGOUT0_{rep}"]
                g["AGOUT1"] = g[f"AGOUT1_{rep}"]
                _pooling(nc, tc, g)
                if stage != "pool":
                    for l, inc in ((1, 6), (2, 8)):
                        _layer(nc, tc, g, l, inc, ntpb)
                        if stage == "l1":
                            break
                if stage in ("pool", "l1"):
                    with tc.tile_pool(name="dbg", bufs=1) as sdbg:
                        dbg = sdbg.tile([128, 256], F32R, tag="dbg", name="dbg")
                        src = g["AGOUT0"] if stage == "pool" else g["AGOUT1"][:128, :256]
                        nc.sync.dma_start(out=dbg[:], in_=src[:128, :256])
                        nc.sync.dma_start(out=g["OUT"][:], in_=dbg[:B, :NLAB].bitcast(F32))
                else:
                    _final(nc, tc, g)

    nc.compile()
    return nc


def _pooling(nc, tc, g):
    pp = g["pp"]
    ident, ones1 = g["ident"], g["ones1"]
    with tc.tile_pool(name="pool_ph", bufs=2) as sp, \
         tc.tile_pool(name="pool_ph1", bufs=1) as bp:
        w1s = bp.tile([128, 6, SPAN_H], F32R, tag="w1s", name="w1s")
        nc.sync.dma_start(out=w1s[:], in_=g["W1"][:].rearrange("(a p) h -> p a h", p=128))
        b1s = bp.tile([128, 4], F32, tag="b1s", name="b1s")
        nc.sync.dma_start(out=b1s[:], in_=g["B1C"][:])
        w2s = bp.tile([128, 4, 1], F32R, tag="w2s", name="w2s")
        nc.sync.dma_start(out=w2s[:], in_=g["W2"][:].rearrange("(a p) o -> p a o", p=128))
        masks = bp.tile([E, DPC, L], F32, tag="masks", name="masks")
        nc.sync.dma_start(out=masks[:], in_=g["MASKT"][:])
        eduT = bp.tile([128, 6, NPC], F32R, tag="eduT", name="eduT")

        for d in range(DPC):
            lhTd = sp.tile([128, 6, L], F32R, tag="lhTd", name="lhTd")
            nc.sync.dma_start(out=lhTd[:], in_=g["LHT"][:, d * L:(d + 1) * L]
                              .rearrange("(a p) t -> p a t", p=128))
            lhd = sp.tile([128, 4, D], F32R, tag="lhd", name="lhd")
            nc.sync.dma_start(out=lhd[:], in_=g["LH"][d * L:(d + 1) * L, :]
                              .rearrange("(a p) x -> p a x", p=128))

            h1 = sp.tile([128, 4, L], F32R, tag="h1", name="h1")
            for mc in range(4):
                h1_ps = pp.tile([128, L], F32, space="PSUM", tag="ps", name="ps")
                for a in range(6):
                    nc.tensor.matmul(out=h1_ps[:],
                                     lhsT=w1s[:, a, mc * 128:(mc + 1) * 128],
                                     rhs=lhTd[:, a, :], start=(a == 0), stop=(a == 5))
                nc.scalar.activation(out=h1[:, mc, :], in_=h1_ps[:], func=ActFn.Relu,
                                     bias=b1s[:, mc:mc + 1])
            att_ps = pp.tile([1, L], F32, space="PSUM", tag="ps", name="ps")
            for mc in range(4):
                nc.tensor.matmul(out=att_ps[:], lhsT=w2s[:, mc, :], rhs=h1[:, mc, :],
                                 start=(mc == 0), stop=(mc == 3))
            att_sb = sp.tile([1, L], F32R, tag="attsb", name="attsb")
            nc.vector.tensor_copy(out=att_sb[:], in_=att_ps[:])
            attb_ps = pp.tile([E, L], F32, space="PSUM", tag="ps", name="ps")
            nc.tensor.matmul(out=attb_ps[:], lhsT=ones1[:, :E], rhs=att_sb[:],
                             start=True, stop=True)
            mp = sp.tile([E, L], F32, tag="mp", name="mp")
            nc.vector.tensor_scalar(out=mp[:], in0=masks[:, d, :], scalar1=100000.0,
                                    scalar2=-100000.0, op0=AluOp.mult, op1=AluOp.add)
            logit = sp.tile([E, L], F32, tag="lgt", name="lgt")
            nc.vector.tensor_tensor(out=logit[:], in0=attb_ps[:], in1=mp[:],
                                    op=AluOp.add)
            ex = sp.tile([E, L], F32, tag="exl", name="exl")
            den = sp.tile([E, 1], F32, tag="denl", name="denl")
            nc.scalar.activation(out=ex[:], in_=logit[:], func=ActFn.Exp,
                                 accum_out=den[:])
            rcp = sp.tile([E, 1], F32, tag="rcpl", name="rcpl")
            nc.vector.reciprocal(out=rcp[:], in_=den[:])
            probs = sp.tile([E, L], F32R, tag="prb", name="prb")
            nc.vector.tensor_scalar_mul(out=probs[:], in0=ex[:], scalar1=rcp[:, :1])
            probsT = sp.tile([128, 4, E], F32R, tag="prbT", name="prbT")
            for lc in range(4):
                pt_ps = pp.tile([128, E], F32R, space="PSUM", tag="ps", name="ps")
                nc.tensor.transpose(out=pt_ps[:], in_=probs[:, lc * 128:(lc + 1) * 128],
                                    identity=ident[:E, :E])
                nc.vector.tensor_copy(out=probsT[:, lc, :], in_=pt_ps[:])
            for db in range(6):
                edu_ps = pp.tile([E, 128], F32, space="PSUM", tag="ps", name="ps")
                for lc in range(4):
                    nc.tensor.matmul(out=edu_ps[:], lhsT=probsT[:, lc, :],
                                     rhs=lhd[:, lc, db * 128:(db + 1) * 128],
                                     start=(lc == 0), stop=(lc == 3))
                edu_sb = sp.tile([E, 128], F32R, tag="edusb", name="edusb")
                nc.vector.tensor_copy(out=edu_sb[:], in_=edu_ps[:])
                et_ps = pp.tile([128, E], F32R, space="PSUM", tag="ps", name="ps")
                nc.tensor.transpose(out=et_ps[:], in_=edu_sb[:], identity=ident[:E, :E])
                nc.vector.tensor_copy(out=eduT[:, db, d * E:(d + 1) * E], in_=et_ps[:])
        nc.sync.dma_start(out=g["AGIN0"][:].rearrange("(a p) n -> p a n", p=128),
                          in_=eduT[:])
    if g.get("no_collectives"):
        nc.sync.dma_start(out=g["AGOUT0"][:D, :], in_=g["AGIN0"][:])
    else:
        nc.gpsimd.collective_compute(
            "AllGather", AluOp.bypass, replica_groups=[list(range(NCORES))],
            ins=[g["AGIN0"].opt()], outs=[g["AGOUT0"].opt()],
        )


def _layer(nc, tc, g, l, inc, ntpb):
    pp, ap_ = g["pp"], g["ap_"]
    ident, iota_f = g["ident"], g["iota_f"]
    ones_blk, ones1 = g["ones_blk"], g["ones1"]
    KTAB, VTAB, PAYTAB = g["KTAB"], g["VTAB"], g["PAYTAB"]
    NTb8 = ntpb * 8

    with tc.tile_pool(name=f"lw{l}", bufs=1) as wp, \
         tc.tile_pool(name=f"lp{l}", bufs=1) as bp, \
         tc.tile_pool(name=f"ls{l}", bufs=2) as sp, \
         tc.tile_pool(name=f"lg{l}", bufs=2) as gp:
        # weights
        WQ = wp.tile([128, inc, 128], F32R, tag="WQ", name="WQ")
        nc.sync.dma_start(out=WQ[:], in_=g[f"WQ{l}"][:].rearrange("(a p) d -> p a d", p=128))
        WK = wp.tile([128, inc + 1, 128], F32R, tag="WK", name="WK")
        nc.sync.dma_start(out=WK[:], in_=g[f"WK{l}"][:].rearrange("(a p) d -> p a d", p=128))
        WV = wp.tile([128, inc + 1, 128], F32R, tag="WV", name="WV")
        nc.sync.dma_start(out=WV[:], in_=g[f"WV{l}"][:].rearrange("(a p) d -> p a d", p=128))
        WS = wp.tile([128, inc + 1, 128], F32R, tag="WS", name="WS")
        nc.sync.dma_start(out=WS[:], in_=g[f"WS{l}"][:].rearrange("(a p) d -> p a d", p=128))
        QB = wp.tile([128, 1], F32, tag="QB", name="QB")
        nc.sync.dma_start(out=QB[:], in_=g[f"QB{l}"][:])
        SB = wp.tile([128, 1], F32, tag="SB", name="SB")
        nc.sync.dma_start(out=SB[:], in_=g[f"SB{l}"][:])
        E1WS = wp.tile([128, 8, 128], F32R, tag="E1WS", name="E1WS")
        nc.sync.dma_start(out=E1WS[:], in_=g[f"E1WS{l}"][:].rearrange("(a p) d -> p a d", p=128))
        WRM4 = wp.tile([128, 8, 4], F32R, tag="WRM4", name="WRM4")
        nc.sync.dma_start(out=WRM4[:], in_=g[f"WRM4{l}"][:].rearrange("(a p) k -> p a k", p=128))

        # wvecs: rows of wv4 = [wc, wr, wm, junk]
        wrm = sp.tile([128, 8, 4], F32R, tag="wrm", name="wrm")
        nc.vector.tensor_copy(out=wrm[:], in_=WRM4[:])
        nc.vector.tensor_tensor(out=wrm[:, :, 0], in0=wrm[:, :, 0], in1=wrm[:, :, 3],
                                op=AluOp.add)
        wv_ps = pp.tile([128, 4], F32, space="PSUM", tag="ps", name="ps")
        for a in range(8):
            nc.tensor.matmul(out=wv_ps[:], lhsT=E1WS[:, a, :], rhs=wrm[:, a, :],
                             start=(a == 0), stop=(a == 7))
        wvecs = bp.tile([128, 4], F32R, tag="wvecs", name="wvecs")
        nc.vector.tensor_copy(out=wvecs[:], in_=wv_ps[:])
        wv4_ps = pp.tile([4, 128], F32R, space="PSUM", tag="ps", name="ps")
        nc.tensor.transpose(out=wv4_ps[:], in_=wvecs[:], identity=ident[:])
        wv4 = bp.tile([4, 128], F32R, tag="wv4", name="wv4")
        nc.vector.tensor_copy(out=wv4[:], in_=wv4_ps[:])

        qT = bp.tile([128, N], F32R, tag="qT", name="qT")
        skipT = bp.tile([128, N], F32R, tag="skipT", name="skipT") if l == 1 else None
        s2nat = bp.tile([128, NBLK, 128], F32, tag="s2nat", name="s2nat") if l == 2 else None
        outT = bp.tile([128, N], F32R, tag="outT", name="outT") if l == 1 else None

        # ---- projections ----
        for b in range(NBLK):
            bs = slice(b * 128, (b + 1) * 128)
            xT = sp.tile([128, inc, 128], F32R, tag="xTblk", name="xTblk")
            if l == 1:
                r, h2 = b // 2, b % 2
                nc.sync.dma_start(
                    out=xT[:],
                    in_=g["AGOUT0"][r * D:(r + 1) * D, h2 * 128:(h2 + 1) * 128]
                        .rearrange("(a p) n -> p a n", p=128))
            else:
                nc.sync.dma_start(
                    out=xT[:],
                    in_=g["AGOUT1"][:, bs].rearrange("(a p) n -> p a n", p=128))
            q_ps = pp.tile([128, 128], F32, space="PSUM", tag="ps", name="ps")
            for a in range(inc):
                nc.tensor.matmul(out=q_ps[:], lhsT=WQ[:, a, :], rhs=xT[:, a, :],
                                 start=(a == 0), stop=(a == inc - 1))
            nc.vector.tensor_scalar_add(out=qT[:, bs], in0=q_ps[:], scalar1=QB[:, :1])
            if l == 1:
                s_ps = pp.tile([128, 128], F32, space="PSUM", tag="ps", name="ps")
                for a in range(inc):
                    nc.tensor.matmul(out=s_ps[:], lhsT=WS[:, a, :], rhs=xT[:, a, :],
                                     start=(a == 0), stop=(a == inc - 1))
                nc.vector.tensor_scalar_add(out=skipT[:, bs], in0=s_ps[:], scalar1=SB[:, :1])
            else:
                s_ps = pp.tile([128, 128], F32, space="PSUM", tag="ps", name="ps")
                for a in range(inc):
                    nc.tensor.matmul(out=s_ps[:], lhsT=xT[:, a, :], rhs=WS[:, a, :],
                                     start=(a == 0), stop=False)
                nc.tensor.matmul(out=s_ps[:], lhsT=ones_blk[:], rhs=WS[:, inc, :],
                                 start=False, stop=True)
                nc.vector.tensor_copy(out=s2nat[:, b, :], in_=s_ps[:])
            for nm, W_, tab, dt_ in (("k", WK, KTAB, BF16), ("v", WV, VTAB, F32R)):
                nat_ps = pp.tile([128, 128], F32, space="PSUM", tag="ps", name="ps")
                for a in range(inc):
                    nc.tensor.matmul(out=nat_ps[:], lhsT=xT[:, a, :], rhs=W_[:, a, :],
                                     start=(a == 0), stop=False)
                nc.tensor.matmul(out=nat_ps[:], lhsT=ones_blk[:], rhs=W_[:, inc, :],
                                 start=False, stop=True)
                stg = sp.tile([128, 128], dt_, tag=f"stg{nm}", name=f"stg{nm}")
                nc.vector.tensor_copy(out=stg[:], in_=nat_ps[:])
                nc.sync.dma_start(out=tab[bs, :], in_=stg[:])
            pay = sp.tile([128, 256], BF16, tag="paystg", name="paystg")
            qn_ps = pp.tile([128, 128], F32R, space="PSUM", tag="ps", name="ps")
            nc.tensor.transpose(out=qn_ps[:], in_=qT[:, bs], identity=ident[:])
            nc.vector.tensor_copy(out=pay[:, 0:128], in_=qn_ps[:])
            abc_ps = pp.tile([4, 128], F32, space="PSUM", tag="ps", name="ps")
            nc.tensor.matmul(out=abc_ps[:], lhsT=wvecs[:], rhs=qT[:, bs],
                             start=True, stop=True)
            abc_sb = sp.tile([4, 128], F32R, tag="abcsb", name="abcsb")
            nc.vector.tensor_copy(out=abc_sb[:], in_=abc_ps[:])
            abcT_ps = pp.tile([128, 4], F32R, space="PSUM", tag="ps", name="ps")
            nc.tensor.transpose(out=abcT_ps[:], in_=abc_sb[:], identity=ident[:4, :4])
            nc.vector.tensor_copy(out=pay[:, 128:132], in_=abcT_ps[:])
            nc.vector.memset(pay[:, 132:256], 0.0)
            nc.sync.dma_start(out=PAYTAB[bs, :], in_=pay[:])

        # ---- edges ----
        for b in range(NBLK):
            bs = slice(b * 128, (b + 1) * 128)
            ts_ = slice(b * ntpb, (b + 1) * ntpb)
            is_ = slice(b * NTb8, (b + 1) * NTb8)
            pay_b = gp.tile([128, ntpb, 256], BF16, tag="payb", name="payb")
            nc.gpsimd.dma_gather(pay_b[:], PAYTAB[bs, :], g["sbDSTIDX"][:, is_],
                                 ntpb * 128, ntpb * 128, 256, single_packet=False)
            ke_b = gp.tile([128, ntpb, 128], BF16, tag="keb", name="keb")
            nc.gpsimd.dma_gather(ke_b[:], KTAB[:], g["sbSRCIDX"][:, is_],
                                 ntpb * 128, ntpb * 128, 128, single_packet=False)
            ve_b = gp.tile([128, ntpb, 128], F32R, tag="veb", name="veb")
            nc.gpsimd.dma_gather(ve_b[:], VTAB[:], g["sbSRCIDX"][:, is_],
                                 ntpb * 128, ntpb * 128, 128, single_packet=False)
            prod = gp.tile([128, ntpb, 128], BF16, tag="prod", name="prod")
            nc.vector.tensor_tensor(out=prod[:], in0=pay_b[:, :, 0:128],
                                    in1=ke_b[:], op=AluOp.mult)
            dots = sp.tile([128, ntpb, 1], F32, tag="dots", name="dots")
            nc.vector.tensor_reduce(out=dots[:], in_=prod[:],
                                    axis=mybir.AxisListType.X, op=AluOp.add)
            abcf = sp.tile([128, ntpb, 4], F32, tag="abcf", name="abcf")
            nc.vector.tensor_copy(out=abcf[:], in_=pay_b[:, :, 128:132])
            tA = sp.tile([128, ntpb], F32, tag="tA", name="tA")
            nc.vector.tensor_tensor(out=tA[:], in0=g["sbTVAL"][:, ts_],
                                    in1=abcf[:, :, 1], op=AluOp.mult)
            mB = sp.tile([128, ntpb], F32, tag="mB", name="mB")
            nc.vector.tensor_tensor(out=mB[:], in0=g["sbMVAL"][:, ts_],
                                    in1=abcf[:, :, 2], op=AluOp.mult)
            lg = sp.tile([128, ntpb], F32, tag="lg", name="lg")
            nc.vector.tensor_tensor(out=lg[:], in0=dots[:, :, 0], in1=abcf[:, :, 0],
                                    op=AluOp.add)
            nc.vector.tensor_tensor(out=lg[:], in0=lg[:], in1=tA[:], op=AluOp.add)
            nc.vector.tensor_tensor(out=lg[:], in0=lg[:], in1=mB[:], op=AluOp.add)
            nc.vector.tensor_tensor(out=lg[:], in0=lg[:], in1=g["sbVADD"][:, ts_],
                                    op=AluOp.add)
            exb = sp.tile([128, ntpb], F32, tag="exb", name="exb")
            nc.scalar.activation(out=exb[:], in_=lg[:], func=ActFn.Exp, scale=ISD)

            agg_ps = ap_.tile([128, 128], F32, space="PSUM", tag="aggps", name="aggps")
            sums_ps = ap_.tile([4, 128], F32, space="PSUM", tag="sumsps", name="sumsps")
            for t in range(ntpb):
                s_t = sp.tile([128, 128], F32R, tag="stile", name="stile")
                nc.vector.tensor_scalar(
                    out=s_t[:], in0=iota_f[:],
                    scalar1=g["sbDSTLOC"][:, b * ntpb + t: b * ntpb + t + 1],
                    scalar2=exb[:, t:t + 1],
                    op0=AluOp.is_equal, op1=AluOp.mult)
                if l == 1:
                    nc.tensor.matmul(out=agg_ps[:], lhsT=ve_b[:, t, :], rhs=s_t[:],
                                     start=(t == 0), stop=False)
                else:
                    nc.tensor.matmul(out=agg_ps[:], lhsT=s_t[:], rhs=ve_b[:, t, :],
                                     start=(t == 0), stop=False)
                nc.tensor.matmul(out=sums_ps[:], lhsT=g["sbETS"][:, b * ntpb + t, :],
                                 rhs=s_t[:], start=(t == 0), stop=(t == ntpb - 1))
            sums_sb = sp.tile([4, 128], F32R, tag="sumssb", name="sumssb")
            nc.vector.tensor_copy(out=sums_sb[:], in_=sums_ps[:])
            if l == 1:
                nc.tensor.matmul(out=agg_ps[:], lhsT=wv4[:], rhs=sums_sb[:],
                                 start=False, stop=True)
            else:
                nc.tensor.matmul(out=agg_ps[:], lhsT=sums_sb[:], rhs=wv4[:],
                                 start=False, stop=True)
            deng = sp.tile([2, 128], F32, tag="deng", name="deng")
            nc.vector.tensor_scalar_max(out=deng[:], in0=sums_sb[:2, :], scalar1=1e-30)
            rcpr = sp.tile([2, 128], F32R, tag="rcpr", name="rcpr")
            with nc.allow_low_precision(reason="f32r recip feeds f32r matmul; ~1e-4 rel"):
                nc.vector.reciprocal(out=rcpr[:], in_=deng[:])
            if l == 1:
                rb_ps = pp.tile([128, 128], F32, space="PSUM", tag="ps", name="ps")
                nc.tensor.matmul(out=rb_ps[:], lhsT=ones1[:], rhs=rcpr[:1, :],
                                 start=True, stop=True)
                rb_sb = sp.tile([128, 128], F32, tag="rbsb", name="rbsb")
                nc.vector.tensor_copy(out=rb_sb[:], in_=rb_ps[:])
                tmp = sp.tile([128, 128], F32, tag="cmb", name="cmb")
                nc.vector.tensor_tensor(out=tmp[:], in0=agg_ps[:], in1=rb_sb[:],
                                        op=AluOp.mult)
                nc.vector.tensor_tensor(out=outT[:, bs], in0=tmp[:], in1=skipT[:, bs],
                                        op=AluOp.add)
            else:
                rc_ps = pp.tile([128, 2], F32R, space="PSUM", tag="ps", name="ps")
                nc.tensor.transpose(out=rc_ps[:], in_=rcpr[:], identity=ident[:2, :2])
                rc_sb = sp.tile([128, 1], F32, tag="rcsb", name="rcsb")
                nc.vector.tensor_copy(out=rc_sb[:], in_=rc_ps[:, :1])
                tmp = sp.tile([128, 128], F32, tag="cmb", name="cmb")
                nc.vector.tensor_scalar_mul(out=tmp[:], in0=agg_ps[:],
                                            scalar1=rc_sb[:, :1])
                o2 = sp.tile([128, 128], F32R, tag="o2", name="o2")
                nc.vector.tensor_tensor(out=o2[:], in0=tmp[:], in1=s2nat[:, b, :],
                                        op=AluOp.add)
                nc.sync.dma_start(out=g["OUT2TAB"][bs, :], in_=o2[:])
        if l == 1:
            nc.sync.dma_start(out=g["AGIN1"][:], in_=outT[:])
    if l == 1:
        if g.get("no_collectives"):
            nc.sync.dma_start(out=g["AGOUT1"][:128, :], in_=g["AGIN1"][:])
        else:
            nc.gpsimd.collective_compute(
                "AllGather", AluOp.bypass, replica_groups=[list(range(NCORES))],
                ins=[g["AGIN1"].opt()], outs=[g["AGOUT1"].opt()],
            )


def _final(nc, tc, g):
    pp = g["pp"]
    with tc.tile_pool(name="fin", bufs=1) as sp:
        sel = sp.tile([128, 1, 128], F32R, tag="sel", name="sel")
        nc.gpsimd.dma_gather(sel[:], g["OUT2TAB"][:], g["sbSELIDX"][:], 128, 128, 128)
        selT_ps = pp.tile([128, 128], F32R, space="PSUM", tag="ps", name="ps")
        nc.tensor.transpose(out=selT_ps[:], in_=sel[:, 0, :], identity=g["ident"][:])
        selT_sb = sp.tile([128, 128], F32R, tag="selTsb", name="selTsb")
        nc.vector.tensor_copy(out=selT_sb[:], in_=selT_ps[:])
        fc_ps = pp.tile([128, NLAB], F32, space="PSUM", tag="ps", name="ps")
        nc.tensor.matmul(out=fc_ps[:], lhsT=selT_sb[:], rhs=g["sbFCW"][:],
                         start=True, stop=False)
        nc.tensor.matmul(out=fc_ps[:], lhsT=g["ones1"][:], rhs=g["sbFCB"][:],
                         start=False, stop=True)
        fc_sb = sp.tile([128, NLAB], F32, tag="fcsb", name="fcsb")
        nc.vector.tensor_copy(out=fc_sb[:], in_=fc_ps[:])
        nc.sync.dma_start(out=g["OUT"][:], in_=fc_sb[:B, :])


# --------------------------------------------------------------------------
# host side
# --------------------------------------------------------------------------

_CACHE = {}


def get_program(ntpb, repeat=1, inline_inputs=None):
    key = (ntpb, repeat, inline_inputs is not None)
    if key not in _CACHE:
        _CACHE[key] = build_program(ntpb, repeat, inline_inputs)
    return _CACHE[key]


def prepare_in_maps(inputs):
    inp = {k: np.asarray(v) for k, v in inputs.items()}
    lh = inp["last_hidden"].astype(np.float32)
    mask = inp["batch_edu_mask"].astype(np.float32)
    lens = inp["edu_lengths"].astype(np.int64)
    edges = inp["batch_edge"].astype(np.int64)

    cnt = np.bincount(edges[1] // 128, minlength=NBLK)
    ntpb = max(18, int(math.ceil(cnt.max() / 128)))

    ed = _edge_prep(edges, inp["batch_edge_type"], inp["batch_is_main"], ntpb)
    selidx = (np.arange(B) * E + (lens - 1)).astype(np.int16)
    ed["SELIDX"] = _wrap_idx(selidx, pad_to=128)

    b1 = inp["b1"].astype(np.float32)
    common = dict(
        W1=inp["W1"].astype(np.float32),
        B1C=np.ascontiguousarray(b1.reshape(4, 128).T),
        W2=inp["W2"].astype(np.float32),
        **ed,
    )

    in_maps = []
    for c in range(NCORES):
        im = dict(common)
        lhc = lh[c * DPC:(c + 1) * DPC].reshape(DPC * L, D)
        im["LH"] = np.ascontiguousarray(lhc)
        im["LHT"] = np.ascontiguousarray(lhc.T)
        im["MASKT"] = np.ascontiguousarray(
            mask[c * DPC:(c + 1) * DPC].transpose(1, 0, 2))
        hs = slice(c * 128, (c + 1) * 128)
        for l, p in ((1, "1"), (2, "2")):
            im[f"WQ{l}"] = np.ascontiguousarray(inp[f"q{p}W"].astype(np.float32)[:, hs])
            im[f"QB{l}"] = np.ascontiguousarray(
                inp[f"q{p}b"].astype(np.float32)[hs].reshape(128, 1))
            for nm, wk, bk in (("WK", f"k{p}W", f"k{p}b"),
                               ("WV", f"v{p}W", f"v{p}b"),
                               ("WS", f"s{p}W", f"s{p}b")):
                w = inp[wk].astype(np.float32)[:, hs]
                ext = np.zeros((w.shape[0] + 128, 128), np.float32)
                ext[:w.shape[0]] = w
                ext[w.shape[0]] = inp[bk].astype(np.float32)[hs]
                im[f"{nm}{l}"] = ext
            im[f"SB{l}"] = np.ascontiguousarray(
                inp[f"s{p}b"].astype(np.float32)[hs].reshape(128, 1))
            im[f"E1WS{l}"] = np.ascontiguousarray(
                inp[f"e{p}W"].astype(np.float32)[:, hs])
            wr = inp[f"Wr{p}"].astype(np.float32).reshape(-1)
            wm = inp[f"Wm{p}"].astype(np.float32).reshape(-1)
            br = inp[f"br{p}"].astype(np.float32)
            bm = inp[f"bm{p}"].astype(np.float32)
            im[f"WRM4{l}"] = np.ascontiguousarray(
                np.stack([br, wr, wm, bm], axis=1).astype(np.float32))
        im["FCW"] = np.ascontiguousarray(inp["fcW"].astype(np.float32)[hs, :])
        fcb = inp["fcb"].astype(np.float32).reshape(1, NLAB)
        im["FCB"] = fcb if c == 0 else np.zeros_like(fcb)
        in_maps.append(im)
    return in_maps, ntpb


def run(inputs, repeat=1):
    in_maps, ntpb = prepare_in_maps(inputs)
    nc = get_program(ntpb, repeat)
    res = run_bass_kernel_spmd(nc, in_maps, list(range(NCORES)))
    out = np.zeros((B, NLAB), np.float64)
    for c in range(NCORES):
        out += res.results[c]["OUT"].astype(np.float64)
    return out.astype(np.float32)


def kernel(**inputs) -> np.ndarray:
    return run(inputs)


def build_timing(ntpb, iters, inline_inputs):
    import os
    """Same body inside a hardware For_i loop, collectives replaced by DMA
    copies. Two builds with different `iters` have byte-identical size, so
    wall-time difference isolates pure body execution."""
    nc = bacc.Bacc("TRN2", target_bir_lowering=False, debug=False)
    NT = NBLK * ntpb

    def din(name, shape, dt):
        data = np.asarray(inline_inputs[name]).astype(mybir.dt.np(dt))
        h = nc.inline_tensor(data, name=name)
        if h.dtype != dt:
            h = bass.DRamTensorHandle(h.name, list(shape), dt)
        return h

    io = dict(
        LHT=din("LHT", [D, DPC * L], F32R),
        LH=din("LH", [DPC * L, D], F32R),
        MASKT=din("MASKT", [E, DPC, L], F32),
        W1=din("W1", [D, SPAN_H], F32R),
        B1C=din("B1C", [128, 4], F32),
        W2=din("W2", [SPAN_H, 1], F32R),
        DSTLOC=din("DSTLOC", [128, NT], F32),
        TVAL=din("TVAL", [128, NT], F32),
        MVAL=din("MVAL", [128, NT], F32),
        VADD=din("VADD", [128, NT], F32),
        ETS=din("ETS", [128, NT, 4], F32R),
        SRCIDX=din("SRCIDX", [128, NT * 8], I16),
        DSTIDX=din("DSTIDX", [128, NT * 8], I16),
        SELIDX=din("SELIDX", [128, 8], I16),
        FCW=din("FCW", [128, NLAB], F32R),
        FCB=din("FCB", [1, NLAB], F32R),
        OUT=nc.dram_tensor("OUT", [B, NLAB], F32, kind="ExternalOutput"),
    )
    for l, inc in ((1, 6), (2, 8)):
        for nm, shp in (("WQ", [inc * 128, 128]), ("QB", [128, 1]),
                        ("WK", [(inc + 1) * 128, 128]),
                        ("WV", [(inc + 1) * 128, 128]),
                        ("WS", [(inc + 1) * 128, 128]), ("SB", [128, 1]),
                        ("E1WS", [C1, 128]), ("WRM4", [C1, 4])):
            dt_ = F32 if nm in ("QB", "SB") else F32R
            io[f"{nm}{l}"] = din(f"{nm}{l}", shp, dt_)

    with tile.TileContext(nc) as tc:
        with (
            tc.tile_pool(name="const", bufs=1) as cp,
            tc.tile_pool(name="ps", bufs=4, space="PSUM") as pp,
            tc.tile_pool(name="acc", bufs=2, space="PSUM") as ap_,
            tc.tile_pool(name="dram", bufs=1, space="DRAM") as dp,
        ):
            g = dict(io)
            g["pp"], g["ap_"], g["cp"], g["tc"] = pp, ap_, cp, tc
            g["no_collectives"] = True

            ident_f = cp.tile([128, 128], F32, tag="identf", name="identf")
            make_identity(nc, ident_f[:])
            g["ident"] = cp.tile([128, 128], F32R, tag="ident", name="ident")
            nc.vector.tensor_copy(out=g["ident"][:], in_=ident_f[:])
            iota_i = cp.tile([128, 128], I32, tag="iotai", name="iotai")
            nc.gpsimd.iota(iota_i[:], pattern=[[1, 128]], base=0,
                           channel_multiplier=0)
            g["iota_f"] = cp.tile([128, 128], F32, tag="iotaf", name="iotaf")
            nc.vector.tensor_copy(out=g["iota_f"][:], in_=iota_i[:])
            ones_f = cp.tile([128, 128], F32, tag="onesf", name="onesf")
            nc.vector.memset(ones_f[:], 0.0)
            nc.vector.memset(ones_f[:1, :], 1.0)
            g["ones_blk"] = cp.tile([128, 128], F32R, tag="onesblk", name="onesblk")
            nc.vector.tensor_copy(out=g["ones_blk"][:], in_=ones_f[:])
            g["ones1"] = cp.tile([1, 128], F32R, tag="ones1", name="ones1")
            nc.vector.tensor_copy(out=g["ones1"][:], in_=ones_f[:1, :])

            g["AGIN0"] = dp.tile([D, NPC], F32R, tag="AGIN0", name="AGIN0")
            g["AGOUT0"] = dp.tile([NCORES * D, NPC], F32R, tag="AGOUT0", name="AGOUT0")
            g["AGIN1"] = dp.tile([128, N], F32R, tag="AGIN1", name="AGIN1")
            g["AGOUT1"] = dp.tile([C1, N], F32R, tag="AGOUT1", name="AGOUT1")
            g["KTAB"] = dp.tile([N, 128], BF16, tag="KTAB", name="KTAB")
            g["VTAB"] = dp.tile([N, 128], F32R, tag="VTAB", name="VTAB")
            g["PAYTAB"] = dp.tile([N, 256], BF16, tag="PAYTAB", name="PAYTAB")
            g["OUT2TAB"] = dp.tile([N, 128], F32R, tag="OUT2TAB", name="OUT2TAB")

            for nm, shp, dt_ in (("DSTLOC", [128, NT], F32), ("TVAL", [128, NT], F32),
                                 ("MVAL", [128, NT], F32), ("VADD", [128, NT], F32),
                                 ("ETS", [128, NT, 4], F32R),
                                 ("SRCIDX", [128, NT * 8], I16),
                                 ("DSTIDX", [128, NT * 8], I16),
                                 ("SELIDX", [128, 8], I16),
                                 ("FCW", [128, NLAB], F32R),
                                 ("FCB", [1, NLAB], F32R)):
                t_ = cp.tile(shp, dt_, tag=f"sb{nm}", name=f"sb{nm}")
                nc.sync.dma_start(out=t_[:], in_=io[nm][:])
                g[f"sb{nm}"] = t_

            tstage = os.environ.get("K_TSTAGE", "full")
            import os as _os
            with tc.For_i(0, iters, 1):
                if tstage in ("full", "pool"):
                    _pooling(nc, tc, g)
                if tstage in ("full", "l1"):
                    _layer(nc, tc, g, 1, 6, ntpb)
                if tstage in ("full", "l2"):
                    _layer(nc, tc, g, 2, 8, ntpb)
                if tstage in ("full", "fin"):
                    _final(nc, tc, g)

    nc.compile()
    return nc



# revision 3
# speedup vs baseline: 1.9569x; 1.9569x over previous
"""Trainium2 Bass kernel for nn_Discourse (BERT span-pool + 2x TransformerConv GNN).

Sharding: data-parallel over docs for the span pooling (8 docs/core);
head-parallel for the graph convs (head h on core h; C=1024 = 8 heads x 128).
Two on-device AllGathers move node features between the two shardings; each is
split into two half-collectives so the wire time overlaps adjacent compute
(AG0-A flies while docs 4-7 pool; AG1-A flies while dst blocks 8-15 run).

Key algebraic simplification: edge_attr @ eW is rank-2 in the edge scalars
(type, is_main):  e_vec(g) = t_g*wr + m_g*wm + wc  with wr = Wr @ eW,
wm = Wm @ eW, wc = (br+bm) @ eW (computed on device).  The k-side edge term
folds into per-node scalars A=q.wr, B=q.wm, C=q.wc; the v-side term folds
into three rank-1 outer products per node block.

Per-edge work uses dst-sorted edges bucketed by 128-node blocks:
one dma_gather of [k|v] source rows + one of the dst payload [q|C,A,B,1].
Per-edge logits come from a single fused 132-wide multiply+reduce:
sum(pay[0:128]*k) + C*1 + A*t + B*m + 1*vadd  (ETS rows are [1, t, m, vadd],
vadd = -1e5 on pad slots), exp without max-subtraction (|logit| < 0.1 at this
model scale; b2 is dropped since softmax is shift-invariant), then
scatter-softmax/aggregation via one-hot (iota==dst_local)*ex matmuls
accumulated in PSUM.  Everything on the PE path runs in bf16 (f32 PSUM
accumulation); the softmax denominators and the final FC stay f32.
"""

import math

import numpy as np

import concourse.bacc as bacc
import concourse.bass as bass
import concourse.mybir as mybir
import concourse.tile as tile
from concourse.bass_utils import run_bass_kernel_spmd
from concourse.masks import make_identity

F32 = mybir.dt.float32
F32R = mybir.dt.float32r
BF16 = mybir.dt.bfloat16
I16 = mybir.dt.int16
I32 = mybir.dt.int32
NPBF = mybir.dt.np(BF16)

B, E, L, D = 64, 32, 512, 768
C1 = C2 = 1024
H = 8
EG = 32768
SPAN_H = 512
NLAB = 4
N = B * E            # 2048 nodes
NCORES = 8
DPC = B // NCORES    # 8 docs per core
NPC = N // NCORES    # 256 nodes per core (pooling side)
NBLK = N // 128      # 16 node blocks
ISD = 1.0 / math.sqrt(128.0)

AluOp = mybir.AluOpType
ActFn = mybir.ActivationFunctionType


def _wrap_idx(idx, pad_to=None):
    """int16 indices -> [128, n/16] wrapped layout (g at [g%16, g//16]),
    replicated across the 8 gpsimd cores (partition groups of 16)."""
    idx = np.asarray(idx, dtype=np.int16)
    n = idx.shape[0]
    if pad_to is not None and n < pad_to:
        idx = np.concatenate([idx, np.zeros(pad_to - n, np.int16)])
        n = pad_to
    assert n % 16 == 0
    w = idx.reshape(n // 16, 16).T
    return np.tile(w, (8, 1)).copy()


def _edge_prep(batch_edge, batch_edge_type, batch_is_main, ntpb):
    """Sort edges by dst, bucket into 16 blocks of 128 dst nodes, pad each
    block to ntpb tiles of 128 edges. Edge slot s of block b sits at
    [partition s%128, tile b*ntpb + s//128]."""
    src = np.asarray(batch_edge[0], dtype=np.int64)
    dst = np.asarray(batch_edge[1], dtype=np.int64)
    t = np.asarray(batch_edge_type, dtype=np.float32).reshape(-1)
    m = np.asarray(batch_is_main, dtype=np.float32).reshape(-1)

    order = np.argsort(dst, kind="stable")
    src, dst, t, m = src[order], dst[order], t[order], m[order]
    blk = dst // 128

    NT = NBLK * ntpb
    cap = ntpb * 128
    dstloc = np.zeros((128, NT), np.float32)
    ets = np.zeros((128, NT, 4), np.float32)
    srcidx_w = np.zeros((128, NT * 8), np.int16)
    dstloc_w = np.zeros((128, NT * 8), np.int16)

    for b in range(NBLK):
        sel = blk == b
        nb = int(sel.sum())
        assert nb <= cap, f"block {b}: {nb} edges > capacity {cap}"
        sl = np.zeros(cap, np.float32)
        sv = np.zeros(cap, np.int16)
        tv = np.zeros(cap, np.float32)
        mv = np.zeros(cap, np.float32)
        dl16 = np.zeros(cap, np.int16)
        on = np.zeros(cap, np.float32)
        sl[:nb] = (dst[sel] - b * 128).astype(np.float32)
        dl16[:nb] = (dst[sel] - b * 128).astype(np.int16)
        sv[:nb] = src[sel].astype(np.int16)
        tv[:nb] = t[sel]
        mv[:nb] = m[sel]
        on[:nb] = 1.0
        va = np.full(cap, -100000.0, np.float32)
        va[:nb] = 0.0
        cols = slice(b * ntpb, (b + 1) * ntpb)
        dstloc[:, cols] = sl.reshape(ntpb, 128).T
        ets[:, cols, 0] = on.reshape(ntpb, 128).T
        ets[:, cols, 1] = tv.reshape(ntpb, 128).T
        ets[:, cols, 2] = mv.reshape(ntpb, 128).T
        ets[:, cols, 3] = va.reshape(ntpb, 128).T
        srcidx_w[:, b * ntpb * 8:(b + 1) * ntpb * 8] = _wrap_idx(sv)
        dstloc_w[:, b * ntpb * 8:(b + 1) * ntpb * 8] = _wrap_idx(dl16)

    return dict(DSTLOC=dstloc, ETS=ets.astype(NPBF),
                SRCIDX=srcidx_w, DSTIDX=dstloc_w)


# --------------------------------------------------------------------------
# device program
# --------------------------------------------------------------------------

def build_program(ntpb, repeat=1):
    nc = bacc.Bacc("TRN2", target_bir_lowering=False, debug=False)
    NT = NBLK * ntpb

    def din(name, shape, dt):
        return nc.dram_tensor(name, shape, dt, kind="ExternalInput")

    io = dict(
        LHT=din("LHT", [D, DPC * L], BF16),
        LH=din("LH", [DPC * L, D], BF16),
        MASKT=din("MASKT", [E, DPC, L], F32),
        W1=din("W1", [D, SPAN_H], BF16),
        B1C=din("B1C", [128, 4], F32),
        W2=din("W2", [SPAN_H, 1], BF16),
        DSTLOC=din("DSTLOC", [128, NT], F32),
        ETS=din("ETS", [128, NT, 4], BF16),
        SRCIDX=din("SRCIDX", [128, NT * 8], I16),
        DSTIDX=din("DSTIDX", [128, NT * 8], I16),
        SELIDX=din("SELIDX", [128, 8], I16),
        FCW=din("FCW", [128, NLAB], F32R),
        FCB=din("FCB", [1, NLAB], F32R),
        OUT=nc.dram_tensor("OUT", [B, NLAB], F32, kind="ExternalOutput"),
    )
    for l, inc in ((1, 6), (2, 8)):
        io[f"WQ{l}"] = din(f"WQ{l}", [inc * 128, 128], BF16)
        io[f"QB{l}"] = din(f"QB{l}", [128, 1], F32)
        io[f"WK{l}"] = din(f"WK{l}", [(inc + 1) * 128, 128], BF16)
        io[f"WV{l}"] = din(f"WV{l}", [(inc + 1) * 128, 128], BF16)
        io[f"WS{l}"] = din(f"WS{l}", [(inc + 1) * 128, 128], BF16)
        io[f"SB{l}"] = din(f"SB{l}", [128, 1], F32)
        io[f"E1WS{l}"] = din(f"E1WS{l}", [C1, 128], BF16)
        io[f"WRM4{l}"] = din(f"WRM4{l}", [C1, 4], BF16)

    with tile.TileContext(nc) as tc:
        with (
            tc.tile_pool(name="const", bufs=1) as cp,
            tc.tile_pool(name="ps", bufs=4, space="PSUM") as pp,
            tc.tile_pool(name="acc", bufs=2, space="PSUM") as ap_,
            tc.tile_pool(name="dram", bufs=1, space="DRAM") as dp,
        ):
            g = dict(io)
            g["pp"], g["ap_"], g["cp"], g["tc"] = pp, ap_, cp, tc

            # constants
            ident_f = cp.tile([128, 128], F32, tag="identf", name="identf")
            make_identity(nc, ident_f[:])
            g["ident"] = cp.tile([128, 128], F32R, tag="ident", name="ident")
            nc.vector.tensor_copy(out=g["ident"][:], in_=ident_f[:])
            g["identb"] = cp.tile([128, 128], BF16, tag="identb", name="identb")
            nc.vector.tensor_copy(out=g["identb"][:], in_=ident_f[:])
            iota_i = cp.tile([128, 128], I32)
            nc.gpsimd.iota(iota_i[:], pattern=[[1, 128]], base=0,
                           channel_multiplier=0)
            g["iota_f"] = cp.tile([128, 128], F32, tag="iotaf", name="iotaf")
            nc.vector.tensor_copy(out=g["iota_f"][:], in_=iota_i[:])
            ones_f = cp.tile([128, 128], F32, tag="onesf", name="onesf")
            nc.vector.memset(ones_f[:], 0.0)
            nc.vector.memset(ones_f[:1, :], 1.0)
            g["ones_blk"] = cp.tile([128, 128], BF16, tag="onesblk", name="onesblk")
            nc.vector.tensor_copy(out=g["ones_blk"][:], in_=ones_f[:])
            g["ones1"] = cp.tile([1, 128], F32R, tag="ones1", name="ones1")
            nc.vector.tensor_copy(out=g["ones1"][:], in_=ones_f[:1, :])
            g["ones1b"] = cp.tile([1, 128], BF16, tag="ones1b", name="ones1b")
            nc.vector.tensor_copy(out=g["ones1b"][:], in_=ones_f[:1, :])

            # persistent DRAM
            g["AGIN0A"] = dp.tile([D, NPC // 2], BF16, tag="AGIN0A", name="AGIN0A")
            g["AGIN0B"] = dp.tile([D, NPC // 2], BF16, tag="AGIN0B", name="AGIN0B")
            g["AGIN1A"] = dp.tile([128, N // 2], BF16, tag="AGIN1A", name="AGIN1A")
            g["AGIN1B"] = dp.tile([128, N // 2], BF16, tag="AGIN1B", name="AGIN1B")
            for _r in range(repeat):
                for nm, shp in (("AGOUT0A", [NCORES * D, NPC // 2]),
                                ("AGOUT0B", [NCORES * D, NPC // 2]),
                                ("AGOUT1A", [C1, N // 2]),
                                ("AGOUT1B", [C1, N // 2])):
                    g[f"{nm}_{_r}"] = dp.tile(
                        shp, BF16, tag=f"{nm}{_r}", name=f"{nm}{_r}",
                        addr_space="Shared")
            g["KVTAB"] = dp.tile([N, 256], BF16, tag="KVTAB", name="KVTAB")
            g["PAYTAB"] = dp.tile([N, 256], BF16, tag="PAYTAB", name="PAYTAB")
            g["OUT2TAB"] = dp.tile([N, 128], F32R, tag="OUT2TAB", name="OUT2TAB")

            # edge arrays (persistent SBUF)
            for nm, shp, dt_ in (("DSTLOC", [128, NT], F32),
                                 ("ETS", [128, NT, 4], BF16),
                                 ("SRCIDX", [128, NT * 8], I16),
                                 ("DSTIDX", [128, NT * 8], I16),
                                 ("SELIDX", [128, 8], I16),
                                 ("FCW", [128, NLAB], F32R),
                                 ("FCB", [1, NLAB], F32R)):
                t_ = cp.tile(shp, dt_, tag=f"sb{nm}", name=f"sb{nm}")
                nc.sync.dma_start(out=t_[:], in_=io[nm][:])
                g[f"sb{nm}"] = t_

            for rep in range(repeat):
                for nm in ("AGOUT0A", "AGOUT0B", "AGOUT1A", "AGOUT1B"):
                    g[nm] = g[f"{nm}_{rep}"]
                _pooling(nc, tc, g)
                for l, inc in ((1, 6), (2, 8)):
                    _layer(nc, tc, g, l, inc, ntpb)
                _final(nc, tc, g)

    nc.compile()
    return nc


def _ag(nc, g, ain, aout):
    nc.gpsimd.collective_compute(
        "AllGather", AluOp.bypass, replica_groups=[list(range(NCORES))],
        ins=[g[ain].opt()], outs=[g[aout].opt()],
    )


def _pooling(nc, tc, g):
    pp = g["pp"]
    identb, ones1b = g["identb"], g["ones1b"]
    with tc.tile_pool(name="pool_ph", bufs=2) as sp, \
         tc.tile_pool(name="pool_ph1", bufs=1) as bp:
        w1s = bp.tile([128, 6, SPAN_H], BF16, tag="w1s", name="w1s")
        nc.sync.dma_start(out=w1s[:], in_=g["W1"][:].rearrange("(a p) h -> p a h", p=128))
        b1s = bp.tile([128, 4], F32, tag="b1s", name="b1s")
        nc.sync.dma_start(out=b1s[:], in_=g["B1C"][:])
        w2s = bp.tile([128, 4, 1], BF16, tag="w2s", name="w2s")
        nc.sync.dma_start(out=w2s[:], in_=g["W2"][:].rearrange("(a p) o -> p a o", p=128))
        masks = bp.tile([E, DPC, L], F32, tag="masks", name="masks")
        nc.sync.dma_start(out=masks[:], in_=g["MASKT"][:])
        eduTA = bp.tile([128, 6, NPC // 2], BF16, tag="eduTA", name="eduTA")
        eduTB = bp.tile([128, 6, NPC // 2], BF16, tag="eduTB", name="eduTB")

        for d in range(DPC):
            half, dh = (eduTA, d) if d < 4 else (eduTB, d - 4)
            lhTd = sp.tile([128, 6, L], BF16, tag="lhTd", name="lhTd")
            nc.sync.dma_start(out=lhTd[:], in_=g["LHT"][:, d * L:(d + 1) * L]
                              .rearrange("(a p) t -> p a t", p=128))
            lhd = sp.tile([128, 4, D], BF16, tag="lhd", name="lhd")
            nc.sync.dma_start(out=lhd[:], in_=g["LH"][d * L:(d + 1) * L, :]
                              .rearrange("(a p) x -> p a x", p=128))

            h1 = sp.tile([128, 4, L], BF16, tag="h1", name="h1")
            for mc in range(4):
                h1_ps = pp.tile([128, L], F32, space="PSUM", tag="ps", name="ps")
                for a in range(6):
                    nc.tensor.matmul(out=h1_ps[:],
                                     lhsT=w1s[:, a, mc * 128:(mc + 1) * 128],
                                     rhs=lhTd[:, a, :], start=(a == 0), stop=(a == 5))
                nc.scalar.activation(out=h1[:, mc, :], in_=h1_ps[:], func=ActFn.Relu,
                                     bias=b1s[:, mc:mc + 1])
            att_ps = pp.tile([1, L], F32, space="PSUM", tag="ps", name="ps")
            for mc in range(4):
                nc.tensor.matmul(out=att_ps[:], lhsT=w2s[:, mc, :], rhs=h1[:, mc, :],
                                 start=(mc == 0), stop=(mc == 3))
            att_sb = sp.tile([1, L], BF16, tag="attsb", name="attsb")
            nc.vector.tensor_copy(out=att_sb[:], in_=att_ps[:])
            attb_ps = pp.tile([E, L], F32, space="PSUM", tag="ps", name="ps")
            nc.tensor.matmul(out=attb_ps[:], lhsT=ones1b[:, :E], rhs=att_sb[:],
                             start=True, stop=True)
            mp = sp.tile([E, L], F32, tag="mp", name="mp")
            nc.vector.tensor_scalar(out=mp[:], in0=masks[:, d, :], scalar1=100000.0,
                                    scalar2=-100000.0, op0=AluOp.mult, op1=AluOp.add)
            logit = sp.tile([E, L], F32, tag="lgt", name="lgt")
            nc.vector.tensor_tensor(out=logit[:], in0=attb_ps[:], in1=mp[:],
                                    op=AluOp.add)
            ex = sp.tile([E, L], F32, tag="exl", name="exl")
            den = sp.tile([E, 1], F32, tag="denl", name="denl")
            nc.scalar.activation(out=ex[:], in_=logit[:], func=ActFn.Exp,
                                 accum_out=den[:])
            rcp = sp.tile([E, 1], F32, tag="rcpl", name="rcpl")
            nc.vector.reciprocal(out=rcp[:], in_=den[:])
            probs = sp.tile([E, L], BF16, tag="prb", name="prb")
            nc.vector.tensor_scalar_mul(out=probs[:], in0=ex[:], scalar1=rcp[:, :1])
            probsT = sp.tile([128, 4, E], BF16, tag="prbT", name="prbT")
            for lc in range(4):
                pt_ps = pp.tile([128, E], BF16, space="PSUM", tag="ps", name="ps")
                nc.tensor.transpose(out=pt_ps[:], in_=probs[:, lc * 128:(lc + 1) * 128],
                                    identity=identb[:E, :E])
                nc.vector.tensor_copy(out=probsT[:, lc, :], in_=pt_ps[:])
            for db in range(6):
                edu_ps = pp.tile([E, 128], F32, space="PSUM", tag="ps", name="ps")
                for lc in range(4):
                    nc.tensor.matmul(out=edu_ps[:], lhsT=probsT[:, lc, :],
                                     rhs=lhd[:, lc, db * 128:(db + 1) * 128],
                                     start=(lc == 0), stop=(lc == 3))
                edu_sb = sp.tile([E, 128], BF16, tag="edusb", name="edusb")
                nc.vector.tensor_copy(out=edu_sb[:], in_=edu_ps[:])
                et_ps = pp.tile([128, E], BF16, space="PSUM", tag="ps", name="ps")
                nc.tensor.transpose(out=et_ps[:], in_=edu_sb[:], identity=identb[:E, :E])
                nc.vector.tensor_copy(out=half[:, db, dh * E:(dh + 1) * E], in_=et_ps[:])
            if d == 3:
                nc.sync.dma_start(
                    out=g["AGIN0A"][:].rearrange("(a p) n -> p a n", p=128),
                    in_=eduTA[:])
                _ag(nc, g, "AGIN0A", "AGOUT0A")
        nc.sync.dma_start(out=g["AGIN0B"][:].rearrange("(a p) n -> p a n", p=128),
                          in_=eduTB[:])
        _ag(nc, g, "AGIN0B", "AGOUT0B")


def _layer(nc, tc, g, l, inc, ntpb):
    pp, ap_ = g["pp"], g["ap_"]
    ident, identb, iota_f = g["ident"], g["identb"], g["iota_f"]
    ones_blk, ones1 = g["ones_blk"], g["ones1"]
    KVTAB, PAYTAB = g["KVTAB"], g["PAYTAB"]
    NTb8 = ntpb * 8

    with tc.tile_pool(name=f"lw{l}", bufs=1) as wp, \
         tc.tile_pool(name=f"lp{l}", bufs=1) as bp, \
         tc.tile_pool(name=f"ls{l}", bufs=2) as sp, \
         tc.tile_pool(name=f"lt{l}", bufs=4) as tp, \
         tc.tile_pool(name=f"lg{l}", bufs=2) as gp:
        # weights
        WQ = wp.tile([128, inc, 128], BF16, tag="WQ", name="WQ")
        nc.sync.dma_start(out=WQ[:], in_=g[f"WQ{l}"][:].rearrange("(a p) d -> p a d", p=128))
        WK = wp.tile([128, inc + 1, 128], BF16, tag="WK", name="WK")
        nc.sync.dma_start(out=WK[:], in_=g[f"WK{l}"][:].rearrange("(a p) d -> p a d", p=128))
        WV = wp.tile([128, inc + 1, 128], BF16, tag="WV", name="WV")
        nc.sync.dma_start(out=WV[:], in_=g[f"WV{l}"][:].rearrange("(a p) d -> p a d", p=128))
        WS = wp.tile([128, inc + 1, 128], BF16, tag="WS", name="WS")
        nc.sync.dma_start(out=WS[:], in_=g[f"WS{l}"][:].rearrange("(a p) d -> p a d", p=128))
        QB = wp.tile([128, 1], F32, tag="QB", name="QB")
        nc.sync.dma_start(out=QB[:], in_=g[f"QB{l}"][:])
        SB = wp.tile([128, 1], F32, tag="SB", name="SB")
        nc.sync.dma_start(out=SB[:], in_=g[f"SB{l}"][:])
        E1WS = wp.tile([128, 8, 128], BF16, tag="E1WS", name="E1WS")
        nc.sync.dma_start(out=E1WS[:], in_=g[f"E1WS{l}"][:].rearrange("(a p) d -> p a d", p=128))
        WRM4 = wp.tile([128, 8, 4], BF16, tag="WRM4", name="WRM4")
        nc.sync.dma_start(out=WRM4[:], in_=g[f"WRM4{l}"][:].rearrange("(a p) k -> p a k", p=128))

        # wvecs: rows of wv4 = [wc, wr, wm, junk]
        wrm = sp.tile([128, 8, 4], BF16, tag="wrm", name="wrm")
        nc.vector.tensor_copy(out=wrm[:], in_=WRM4[:])
        nc.vector.tensor_tensor(out=wrm[:, :, 0], in0=wrm[:, :, 0], in1=wrm[:, :, 3],
                                op=AluOp.add)
        wv_ps = pp.tile([128, 4], F32, space="PSUM", tag="ps", name="ps")
        for a in range(8):
            nc.tensor.matmul(out=wv_ps[:], lhsT=E1WS[:, a, :], rhs=wrm[:, a, :],
                             start=(a == 0), stop=(a == 7))
        wvecs = bp.tile([128, 4], BF16, tag="wvecs", name="wvecs")
        nc.vector.tensor_copy(out=wvecs[:], in_=wv_ps[:])
        wv4_ps = pp.tile([4, 128], BF16, space="PSUM", tag="ps", name="ps")
        nc.tensor.transpose(out=wv4_ps[:], in_=wvecs[:], identity=identb[:])
        wv4 = bp.tile([4, 128], BF16, tag="wv4", name="wv4")
        nc.vector.tensor_copy(out=wv4[:], in_=wv4_ps[:])

        qT = bp.tile([128, N], BF16, tag="qT", name="qT")
        skipT = bp.tile([128, N], BF16, tag="skipT", name="skipT") if l == 1 else None
        s2nat = bp.tile([128, NBLK, 128], BF16, tag="s2nat", name="s2nat") if l == 2 else None
        outTA = bp.tile([128, N // 2], BF16, tag="outTA", name="outTA") if l == 1 else None
        outTB = bp.tile([128, N // 2], BF16, tag="outTB", name="outTB") if l == 1 else None

        # ---- projections ----
        # l1: even blocks depend on AG0-A, odd on AG0-B; l2: blocks 0-7 on
        # AG1-A, 8-15 on AG1-B.  Ordered so the first half starts while the
        # second half-collective is still in flight.
        if l == 1:
            border = [b for b in range(NBLK) if b % 2 == 0] + \
                     [b for b in range(NBLK) if b % 2 == 1]
        else:
            border = list(range(NBLK))
        for b in border:
            bs = slice(b * 128, (b + 1) * 128)
            xT = sp.tile([128, inc, 128], BF16, tag="xTblk", name="xTblk")
            if l == 1:
                r, h2 = b // 2, b % 2
                src_buf = g["AGOUT0A"] if h2 == 0 else g["AGOUT0B"]
                nc.sync.dma_start(
                    out=xT[:],
                    in_=src_buf[r * D:(r + 1) * D, :]
                        .rearrange("(a p) n -> p a n", p=128))
            else:
                src_buf = g["AGOUT1A"] if b < 8 else g["AGOUT1B"]
                cs = slice((b % 8) * 128, (b % 8 + 1) * 128)
                nc.sync.dma_start(
                    out=xT[:],
                    in_=src_buf[:, cs].rearrange("(a p) n -> p a n", p=128))
            q_ps = pp.tile([128, 128], F32, space="PSUM", tag="ps", name="ps")
            for a in range(inc):
                nc.tensor.matmul(out=q_ps[:], lhsT=WQ[:, a, :], rhs=xT[:, a, :],
                                 start=(a == 0), stop=(a == inc - 1))
            nc.vector.tensor_scalar_add(out=qT[:, bs], in0=q_ps[:], scalar1=QB[:, :1])
            if l == 1:
                s_ps = pp.tile([128, 128], F32, space="PSUM", tag="ps", name="ps")
                for a in range(inc):
                    nc.tensor.matmul(out=s_ps[:], lhsT=WS[:, a, :], rhs=xT[:, a, :],
                                     start=(a == 0), stop=(a == inc - 1))
                nc.vector.tensor_scalar_add(out=skipT[:, bs], in0=s_ps[:], scalar1=SB[:, :1])
            else:
                s_ps = pp.tile([128, 128], F32, space="PSUM", tag="ps", name="ps")
                for a in range(inc):
                    nc.tensor.matmul(out=s_ps[:], lhsT=xT[:, a, :], rhs=WS[:, a, :],
                                     start=(a == 0), stop=False)
                nc.tensor.matmul(out=s_ps[:], lhsT=ones_blk[:], rhs=WS[:, inc, :],
                                 start=False, stop=True)
                nc.vector.tensor_copy(out=s2nat[:, b, :], in_=s_ps[:])
            kv = sp.tile([128, 256], BF16, tag="kvstg", name="kvstg")
            for ci, (W_,) in enumerate(((WK,), (WV,))):
                nat_ps = pp.tile([128, 128], F32, space="PSUM", tag="ps", name="ps")
                for a in range(inc):
                    nc.tensor.matmul(out=nat_ps[:], lhsT=xT[:, a, :], rhs=W_[:, a, :],
                                     start=(a == 0), stop=False)
                nc.tensor.matmul(out=nat_ps[:], lhsT=ones_blk[:], rhs=W_[:, inc, :],
                                 start=False, stop=True)
                nc.vector.tensor_copy(out=kv[:, ci * 128:(ci + 1) * 128], in_=nat_ps[:])
            nc.sync.dma_start(out=KVTAB[bs, :], in_=kv[:])
            pay = sp.tile([128, 256], BF16, tag="paystg", name="paystg")
            qn_ps = pp.tile([128, 128], BF16, space="PSUM", tag="ps", name="ps")
            nc.tensor.transpose(out=qn_ps[:], in_=qT[:, bs], identity=identb[:])
            nc.vector.tensor_copy(out=pay[:, 0:128], in_=qn_ps[:])
            abc_ps = pp.tile([4, 128], F32, space="PSUM", tag="ps", name="ps")
            nc.tensor.matmul(out=abc_ps[:], lhsT=wvecs[:], rhs=qT[:, bs],
                             start=True, stop=True)
            abc_sb = sp.tile([4, 128], BF16, tag="abcsb", name="abcsb")
            nc.vector.tensor_copy(out=abc_sb[:], in_=abc_ps[:])
            abcT_ps = pp.tile([128, 4], BF16, space="PSUM", tag="ps", name="ps")
            nc.tensor.transpose(out=abcT_ps[:], in_=abc_sb[:], identity=identb[:4, :4])
            nc.vector.tensor_copy(out=pay[:, 128:132], in_=abcT_ps[:])
            nc.vector.memset(pay[:, 131:132], 1.0)
            nc.vector.memset(pay[:, 132:256], 0.0)
            nc.sync.dma_start(out=PAYTAB[bs, :], in_=pay[:])

        # ---- edges ----
        for b in range(NBLK):
            bs = slice(b * 128, (b + 1) * 128)
            ts_ = slice(b * ntpb, (b + 1) * ntpb)
            is_ = slice(b * NTb8, (b + 1) * NTb8)
            pay_b = gp.tile([128, ntpb, 256], BF16, tag="payb", name="payb")
            nc.gpsimd.dma_gather(pay_b[:], PAYTAB[bs, :], g["sbDSTIDX"][:, is_],
                                 ntpb * 128, ntpb * 128, 256, single_packet=False)
            kve_b = gp.tile([128, ntpb, 256], BF16, tag="kveb", name="kveb")
            nc.gpsimd.dma_gather(kve_b[:], KVTAB[:], g["sbSRCIDX"][:, is_],
                                 ntpb * 128, ntpb * 128, 256, single_packet=False)
            prod = gp.tile([128, ntpb, 132], BF16, tag="prod", name="prod")
            nc.vector.tensor_tensor(out=prod[:, :, 0:128], in0=pay_b[:, :, 0:128],
                                    in1=kve_b[:, :, 0:128], op=AluOp.mult)
            nc.vector.tensor_tensor(out=prod[:, :, 128:132], in0=pay_b[:, :, 128:132],
                                    in1=g["sbETS"][:, ts_, :], op=AluOp.mult)
            lg = sp.tile([128, ntpb, 1], F32, tag="lg", name="lg")
            nc.vector.tensor_reduce(out=lg[:], in_=prod[:],
                                    axis=mybir.AxisListType.X, op=AluOp.add)
            exb = sp.tile([128, ntpb], F32, tag="exb", name="exb")
            nc.scalar.activation(out=exb[:], in_=lg[:, :, 0], func=ActFn.Exp, scale=ISD)

            agg_ps = ap_.tile([128, 128], F32, space="PSUM", tag="aggps", name="aggps")
            sums_ps = ap_.tile([4, 128], F32, space="PSUM", tag="sumsps", name="sumsps")
            for t in range(ntpb):
                s_t = tp.tile([128, 128], BF16, tag="stile", name="stile")
                nc.vector.tensor_scalar(
                    out=s_t[:], in0=iota_f[:],
                    scalar1=g["sbDSTLOC"][:, b * ntpb + t: b * ntpb + t + 1],
                    scalar2=exb[:, t:t + 1],
                    op0=AluOp.is_equal, op1=AluOp.mult)
                if l == 1:
                    nc.tensor.matmul(out=agg_ps[:], lhsT=kve_b[:, t, 128:256], rhs=s_t[:],
                                     start=(t == 0), stop=False)
                else:
                    nc.tensor.matmul(out=agg_ps[:], lhsT=s_t[:], rhs=kve_b[:, t, 128:256],
                                     start=(t == 0), stop=False)
                nc.tensor.matmul(out=sums_ps[:], lhsT=g["sbETS"][:, b * ntpb + t, :],
                                 rhs=s_t[:], start=(t == 0), stop=(t == ntpb - 1))
            sums_sb = sp.tile([4, 128], BF16, tag="sumssb", name="sumssb")
            nc.vector.tensor_copy(out=sums_sb[:], in_=sums_ps[:])
            if l == 1:
                nc.tensor.matmul(out=agg_ps[:], lhsT=wv4[:], rhs=sums_sb[:],
                                 start=False, stop=True)
            else:
                nc.tensor.matmul(out=agg_ps[:], lhsT=sums_sb[:], rhs=wv4[:],
                                 start=False, stop=True)
            deng = sp.tile([2, 128], F32, tag="deng", name="deng")
            nc.vector.tensor_scalar_max(out=deng[:], in0=sums_ps[:2, :], scalar1=1e-30)
            rcpr = sp.tile([2, 128], F32R, tag="rcpr", name="rcpr")
            with nc.allow_low_precision(reason="f32r recip feeds f32r matmul; ~1e-4 rel"):
                nc.vector.reciprocal(out=rcpr[:], in_=deng[:])
            if l == 1:
                rb_ps = pp.tile([128, 128], F32, space="PSUM", tag="ps", name="ps")
                nc.tensor.matmul(out=rb_ps[:], lhsT=ones1[:], rhs=rcpr[:1, :],
                                 start=True, stop=True)
                rb_sb = sp.tile([128, 128], F32, tag="rbsb", name="rbsb")
                nc.vector.tensor_copy(out=rb_sb[:], in_=rb_ps[:])
                tmp = sp.tile([128, 128], F32, tag="cmb", name="cmb")
                nc.vector.tensor_tensor(out=tmp[:], in0=agg_ps[:], in1=rb_sb[:],
                                        op=AluOp.mult)
                half, cs = (outTA, b) if b < 8 else (outTB, b - 8)
                nc.vector.tensor_tensor(out=half[:, cs * 128:(cs + 1) * 128],
                                        in0=tmp[:], in1=skipT[:, bs], op=AluOp.add)
                if b == 7:
                    nc.sync.dma_start(out=g["AGIN1A"][:], in_=outTA[:])
                    _ag(nc, g, "AGIN1A", "AGOUT1A")
            else:
                rc_ps = pp.tile([128, 2], F32R, space="PSUM", tag="ps", name="ps")
                nc.tensor.transpose(out=rc_ps[:], in_=rcpr[:], identity=ident[:2, :2])
                rc_sb = sp.tile([128, 1], F32, tag="rcsb", name="rcsb")
                nc.vector.tensor_copy(out=rc_sb[:], in_=rc_ps[:, :1])
                tmp = sp.tile([128, 128], F32, tag="cmb", name="cmb")
                nc.vector.tensor_scalar_mul(out=tmp[:], in0=agg_ps[:],
                                            scalar1=rc_sb[:, :1])
                o2 = sp.tile([128, 128], F32R, tag="o2", name="o2")
                nc.vector.tensor_tensor(out=o2[:], in0=tmp[:], in1=s2nat[:, b, :],
                                        op=AluOp.add)
                nc.sync.dma_start(out=g["OUT2TAB"][bs, :], in_=o2[:])
        if l == 1:
            nc.sync.dma_start(out=g["AGIN1B"][:], in_=outTB[:])
            _ag(nc, g, "AGIN1B", "AGOUT1B")


def _final(nc, tc, g):
    pp = g["pp"]
    with tc.tile_pool(name="fin", bufs=1) as sp:
        sel = sp.tile([128, 1, 128], F32R, tag="sel", name="sel")
        nc.gpsimd.dma_gather(sel[:], g["OUT2TAB"][:], g["sbSELIDX"][:], 128, 128, 128)
        selT_ps = pp.tile([128, 128], F32R, space="PSUM", tag="ps", name="ps")
        nc.tensor.transpose(out=selT_ps[:], in_=sel[:, 0, :], identity=g["ident"][:])
        selT_sb = sp.tile([128, 128], F32R, tag="selTsb", name="selTsb")
        nc.vector.tensor_copy(out=selT_sb[:], in_=selT_ps[:])
        fc_ps = pp.tile([128, NLAB], F32, space="PSUM", tag="ps", name="ps")
        nc.tensor.matmul(out=fc_ps[:], lhsT=selT_sb[:], rhs=g["sbFCW"][:],
                         start=True, stop=False)
        nc.tensor.matmul(out=fc_ps[:], lhsT=g["ones1"][:], rhs=g["sbFCB"][:],
                         start=False, stop=True)
        fc_sb = sp.tile([128, NLAB], F32, tag="fcsb", name="fcsb")
        nc.vector.tensor_copy(out=fc_sb[:], in_=fc_ps[:])
        nc.sync.dma_start(out=g["OUT"][:], in_=fc_sb[:B, :])


# --------------------------------------------------------------------------
# host side
# --------------------------------------------------------------------------

_CACHE = {}


def get_program(ntpb, repeat=1):
    key = (ntpb, repeat)
    if key not in _CACHE:
        _CACHE[key] = build_program(ntpb, repeat)
    return _CACHE[key]


def prepare_in_maps(inputs):
    inp = {k: np.asarray(v) for k, v in inputs.items()}
    lh = inp["last_hidden"].astype(np.float32)
    mask = inp["batch_edu_mask"].astype(np.float32)
    lens = inp["edu_lengths"].astype(np.int64)
    edges = inp["batch_edge"].astype(np.int64)

    cnt = np.bincount(edges[1] // 128, minlength=NBLK)
    ntpb = max(18, int(math.ceil(cnt.max() / 128)))

    ed = _edge_prep(edges, inp["batch_edge_type"], inp["batch_is_main"], ntpb)
    selidx = (np.arange(B) * E + (lens - 1)).astype(np.int16)
    ed["SELIDX"] = _wrap_idx(selidx, pad_to=128)

    b1 = inp["b1"].astype(np.float32)
    common = dict(
        W1=inp["W1"].astype(NPBF),
        B1C=np.ascontiguousarray(b1.reshape(4, 128).T),
        W2=inp["W2"].astype(NPBF),
        **ed,
    )

    in_maps = []
    for c in range(NCORES):
        im = dict(common)
        lhc = lh[c * DPC:(c + 1) * DPC].reshape(DPC * L, D)
        im["LH"] = np.ascontiguousarray(lhc).astype(NPBF)
        im["LHT"] = np.ascontiguousarray(lhc.T).astype(NPBF)
        im["MASKT"] = np.ascontiguousarray(
            mask[c * DPC:(c + 1) * DPC].transpose(1, 0, 2))
        hs = slice(c * 128, (c + 1) * 128)
        for l, p in ((1, "1"), (2, "2")):
            im[f"WQ{l}"] = np.ascontiguousarray(
                inp[f"q{p}W"].astype(np.float32)[:, hs]).astype(NPBF)
            im[f"QB{l}"] = np.ascontiguousarray(
                inp[f"q{p}b"].astype(np.float32)[hs].reshape(128, 1))
            for nm, wk, bk in (("WK", f"k{p}W", f"k{p}b"),
                               ("WV", f"v{p}W", f"v{p}b"),
                               ("WS", f"s{p}W", f"s{p}b")):
                w = inp[wk].astype(np.float32)[:, hs]
                ext = np.zeros((w.shape[0] + 128, 128), np.float32)
                ext[:w.shape[0]] = w
                ext[w.shape[0]] = inp[bk].astype(np.float32)[hs]
                im[f"{nm}{l}"] = ext.astype(NPBF)
            im[f"SB{l}"] = np.ascontiguousarray(
                inp[f"s{p}b"].astype(np.float32)[hs].reshape(128, 1))
            im[f"E1WS{l}"] = np.ascontiguousarray(
                inp[f"e{p}W"].astype(np.float32)[:, hs]).astype(NPBF)
            wr = inp[f"Wr{p}"].astype(np.float32).reshape(-1)
            wm = inp[f"Wm{p}"].astype(np.float32).reshape(-1)
            br = inp[f"br{p}"].astype(np.float32)
            bm = inp[f"bm{p}"].astype(np.float32)
            im[f"WRM4{l}"] = np.ascontiguousarray(
                np.stack([br, wr, wm, bm], axis=1).astype(np.float32)).astype(NPBF)
        im["FCW"] = np.ascontiguousarray(inp["fcW"].astype(np.float32)[hs, :])
        fcb = inp["fcb"].astype(np.float32).reshape(1, NLAB)
        im["FCB"] = fcb if c == 0 else np.zeros_like(fcb)
        in_maps.append(im)
    return in_maps, ntpb


def run(inputs, repeat=1):
    in_maps, ntpb = prepare_in_maps(inputs)
    nc = get_program(ntpb, repeat)
    res = run_bass_kernel_spmd(nc, in_maps, list(range(NCORES)))
    out = np.zeros((B, NLAB), np.float64)
    for c in range(NCORES):
        out += res.results[c]["OUT"].astype(np.float64)
    return out.astype(np.float32)


def kernel(**inputs) -> np.ndarray:
    return run(inputs)
